# revision 5
# baseline (speedup 1.0000x reference)
"""2-layer LSTM (S=256, B=32, D=H=1024) on 8 trn2 NeuronCores.

Strategy: tensor-parallel over the hidden dim. Core j owns H-slice
[128j:128(j+1)) and, for each layer, the four gate column blocks
(f, i, o, g) of that slice -> 512 local gate columns.

Per layer:
  Phase A/C: xproj = x @ Wx + b as a big GEMM (tokens x 512 per core),
  kept resident in SBUF (bf16).
  Phase B/D: 256 sequential steps. Each step: 8 K-chunk matmuls
  (lhsT = gathered h^T chunks [128,32] bf16, rhs = Wh slice [128,512] bf16)
  accumulate gates in PSUM; DVE adds xproj; ACT sigmoid/tanh; DVE cell
  update (f32); PE-transpose of the local h chunk [32,128]->[128,32];
  AllGather of bf16 chunks across the 8 cores -> next step's h^T.

Layouts: "natural" batch-on-partitions for gates/cell; gathered h^T is
feature-on-partitions, which is exactly the lhsT the next matmul needs.
"""

import numpy as np
import ml_dtypes

S_FULL = 256
B = 32
D = 1024
H = 1024
NC = 8
HL = H // NC          # 128 hidden rows per core
NG = 4 * HL           # 512 local gate columns
KC = D // 128         # 8 contraction chunks

_BF16 = ml_dtypes.bfloat16


def _build(nc_mod, steps):
    import concourse.mybir as mybir
    import concourse.tile as tile

    nc = nc_mod
    f32 = mybir.dt.float32
    bf16 = mybir.dt.bfloat16
    AF = mybir.ActivationFunctionType
    NTOK = steps * B
    MT = NTOK // 128       # token m-tiles (steps/4)

    # ---- DRAM I/O ----
    xT = nc.dram_tensor("xT", [D, NTOK], bf16, kind="ExternalInput")
    wx1 = nc.dram_tensor("wx1", [D, NG], bf16, kind="ExternalInput")
    wx2 = nc.dram_tensor("wx2", [H, NG], bf16, kind="ExternalInput")
    wh1 = nc.dram_tensor("wh1", [H, NG], bf16, kind="ExternalInput")
    wh2 = nc.dram_tensor("wh2", [H, NG], bf16, kind="ExternalInput")
    b1 = nc.dram_tensor("b1", [128, NG], f32, kind="ExternalInput")
    b2 = nc.dram_tensor("b2", [128, NG], f32, kind="ExternalInput")
    ident = nc.dram_tensor("ident", [32, 32], f32, kind="ExternalInput")
    hseq = nc.dram_tensor("hseq", [steps, B, HL], f32, kind="ExternalOutput")
    hcfin = nc.dram_tensor("hcfin", [4, B, HL], f32, kind="ExternalOutput")

    with tile.TileContext(nc) as tc:
        with (
            tc.tile_pool(name="weights", bufs=1) as wpool,
            tc.tile_pool(name="xp", bufs=1) as xppool,
            tc.tile_pool(name="stage", bufs=3) as stpool,
            tc.tile_pool(name="gather", bufs=3) as gpool,
            tc.tile_pool(name="ew", bufs=3) as ewpool,
            tc.tile_pool(name="state", bufs=1) as cpool,
            tc.tile_pool(name="psA", bufs=2, space="PSUM") as psapool,
            tc.tile_pool(name="psG", bufs=2, space="PSUM") as psgpool,
            tc.tile_pool(name="psT", bufs=2, space="PSUM") as pstpool,
            tc.tile_pool(name="dram", bufs=3, space="DRAM") as dpool,
        ):
            # persistent SBUF
            xp_sb = xppool.tile([128, MT * NG], bf16, tag="xp")
            wh1_sb = wpool.tile([128, KC * NG], bf16, tag="wh1")
            wh2_sb = wpool.tile([128, KC * NG], bf16, tag="wh2")
            ident_sb = wpool.tile([32, 32], f32, tag="ident")
            nc.sync.dma_start(ident_sb[:], ident[:])
            nc.sync.dma_start(
                wh1_sb[:].rearrange("p (k g) -> p k g", k=KC),
                wh1.rearrange("(k p) g -> p k g", p=128),
            )
            nc.sync.dma_start(
                wh2_sb[:].rearrange("p (k g) -> p k g", k=KC),
                wh2.rearrange("(k p) g -> p k g", p=128),
            )
            h1T_dram = dpool.tile([KC * 128, NTOK], bf16, tag="h1T")

            def xproj_phase(w_dram, b_dram, src_view):
                """GEMM: xp_sb[:, m*NG:(m+1)*NG] = src^T-tiles @ w + b."""
                w_sb = wpool.tile([128, KC * NG], bf16, tag="wx")
                b_sb = wpool.tile([128, NG], f32, tag="bias")
                nc.sync.dma_start(
                    w_sb[:].rearrange("p (k g) -> p k g", k=KC),
                    w_dram.rearrange("(k p) g -> p k g", p=128),
                )
                nc.sync.dma_start(b_sb[:], b_dram[:])
                for m in range(MT):
                    a_sb = stpool.tile([128, KC * 128], bf16, tag="astage")
                    nc.sync.dma_start(
                        a_sb[:].rearrange("p (k n) -> p k n", k=KC),
                        src_view[:, :, m * 128:(m + 1) * 128],
                    )
                    ps = psapool.tile([128, NG], f32, tag="psA")
                    for k in range(KC):
                        nc.tensor.matmul(
                            ps[:],
                            a_sb[:, k * 128:(k + 1) * 128],
                            w_sb[:, k * NG:(k + 1) * NG],
                            start=(k == 0),
                            stop=(k == KC - 1),
                        )
                    nc.vector.tensor_add(
                        xp_sb[:, m * NG:(m + 1) * NG], ps[:], b_sb[:]
                    )

            def recurrent_phase(layer, wh_sb):
                c_sb = cpool.tile([B, HL], f32, tag=f"c{layer}")
                nc.vector.memset(c_sb[:], 0.0)
                hT_cur = gpool.tile([128, KC * 32], bf16, tag="hT")
                nc.vector.memset(hT_cur[:], 0.0)
                h_last = None
                for t in range(steps):
                    m, r = t // 4, t % 4
                    # gates = h_{t-1} @ Wh  (8 K-chunk matmuls)
                    ps_g = psgpool.tile([B, NG], f32, tag="psG")
                    for k in range(KC):
                        nc.tensor.matmul(
                            ps_g[:],
                            hT_cur[:, k * 32:(k + 1) * 32],
                            wh_sb[:, k * NG:(k + 1) * NG],
                            start=(k == 0),
                            stop=(k == KC - 1),
                        )
                    gates = ewpool.tile([B, NG], f32, tag="gates")
                    nc.vector.tensor_add(
                        gates[:],
                        ps_g[:],
                        xp_sb[32 * r:32 * r + 32, m * NG:(m + 1) * NG],
                    )
                    act = ewpool.tile([B, NG], f32, tag="act")
                    nc.scalar.activation(act[:, 0:384], gates[:, 0:384], AF.Sigmoid)
                    nc.scalar.activation(act[:, 384:512], gates[:, 384:512], AF.Tanh)
                    # f=act[:,0:128] i=act[:,128:256] o=act[:,256:384] g=act[:,384:512]
                    t1 = ewpool.tile([B, HL], f32, tag="t1")
                    t2 = ewpool.tile([B, HL], f32, tag="t2")
                    nc.vector.tensor_mul(t1[:], act[:, 0:128], c_sb[:])
                    nc.vector.tensor_mul(t2[:], act[:, 128:256], act[:, 384:512])
                    nc.vector.tensor_add(c_sb[:], t1[:], t2[:])
                    ct = ewpool.tile([B, HL], f32, tag="ct")
                    nc.scalar.activation(ct[:], c_sb[:], AF.Tanh)
                    h = ewpool.tile([B, HL], f32, tag="h")
                    nc.vector.tensor_mul(h[:], act[:, 256:384], ct[:])
                    h_last = h
                    if layer == 2:
                        nc.sync.dma_start(hseq[t], h[:])
                    last = t == steps - 1
                    if layer == 2 and last:
                        break  # h2 of the final step feeds nothing downstream
                    # h^T chunk -> all-gather -> next step's lhsT chunks
                    tr = pstpool.tile([128, 32], f32, tag="psT")
                    nc.tensor.transpose(tr[:], h[:], ident_sb[:])
                    hTb = ewpool.tile([128, 32], bf16, tag="hTb")
                    nc.vector.tensor_copy(hTb[:], tr[:])
                    bounce = dpool.tile([128, 32], bf16, tag="bounce")
                    gath = dpool.tile([NC * 128, 32], bf16, tag="gath")
                    nc.sync.dma_start(bounce[:], hTb[:])
                    nc.gpsimd.collective_compute(
                        "AllGather",
                        mybir.AluOpType.bypass,
                        replica_groups=[list(range(NC))],
                        ins=[bounce[:].opt()],
                        outs=[gath[:].opt()],
                    )
                    if layer == 1:
                        nc.sync.dma_start(
                            h1T_dram[:].rearrange("(k p) n -> k p n", p=128)[
                                :, :, 32 * t:32 * t + 32
                            ],
                            gath[:].rearrange("(k p) b -> k p b", p=128),
                        )
                    if not last:
                        hT_next = gpool.tile([128, KC * 32], bf16, tag="hT")
                        nc.sync.dma_start(
                            hT_next[:].rearrange("p (k b) -> p k b", k=KC),
                            gath[:].rearrange("(k p) b -> p k b", p=128),
                        )
                        hT_cur = hT_next
                # finals: h / c of the last step
                base = 0 if layer == 1 else 2
                nc.sync.dma_start(hcfin[base], h_last[:])
                nc.sync.dma_start(hcfin[base + 1], c_sb[:])

            # layer 1
            xproj_phase(wx1, b1, xT.rearrange("(k p) n -> p k n", p=128))
            recurrent_phase(1, wh1_sb)
            # layer 2
            xproj_phase(
                wx2, b2, h1T_dram[:].rearrange("(k p) n -> p k n", p=128)
            )
            recurrent_phase(2, wh2_sb)

    return nc


def _run(inputs, steps):
    import concourse.bacc as bacc
    from concourse.bass_utils import run_bass_kernel_spmd

    x = np.asarray(inputs["x"], np.float32)[:steps]
    Wx = np.asarray(inputs["Wx"], np.float32)
    Wh = np.asarray(inputs["Wh"], np.float32)
    b = np.asarray(inputs["b"], np.float32)

    xT = np.ascontiguousarray(
        x.reshape(steps * B, D).T
    ).astype(_BF16)
    ident = np.eye(32, dtype=np.float32)

    in_maps = []
    for j in range(NC):
        # local gate column order: f, i, o, g  (reference order: f, i, g, o)
        idx = np.concatenate(
            [np.arange(g * H + HL * j, g * H + HL * (j + 1)) for g in (0, 1, 3, 2)]
        )
        m = {
            "xT": xT,
            "wx1": np.ascontiguousarray(Wx[0][:, idx]).astype(_BF16),
            "wx2": np.ascontiguousarray(Wx[1][:, idx]).astype(_BF16),
            "wh1": np.ascontiguousarray(Wh[0][:, idx]).astype(_BF16),
            "wh2": np.ascontiguousarray(Wh[1][:, idx]).astype(_BF16),
            "b1": np.ascontiguousarray(
                np.broadcast_to(b[0][idx], (128, NG))
            ).astype(np.float32),
            "b2": np.ascontiguousarray(
                np.broadcast_to(b[1][idx], (128, NG))
            ).astype(np.float32),
            "ident": ident,
        }
        in_maps.append(m)

    nc = bacc.Bacc("TRN2", target_bir_lowering=False, debug=False, num_devices=NC)
    _build(nc, steps)
    nc.compile()
    res = run_bass_kernel_spmd(nc, in_maps, core_ids=list(range(NC)))

    hseq = np.concatenate(
        [res.results[j]["hseq"] for j in range(NC)], axis=2
    ).astype(np.float32)
    hc = [res.results[j]["hcfin"] for j in range(NC)]
    new_h = np.stack(
        [
            np.concatenate([hc[j][0] for j in range(NC)], axis=1),
            np.concatenate([hc[j][2] for j in range(NC)], axis=1),
        ]
    ).astype(np.float32)
    new_c = np.stack(
        [
            np.concatenate([hc[j][1] for j in range(NC)], axis=1),
            np.concatenate([hc[j][3] for j in range(NC)], axis=1),
        ]
    ).astype(np.float32)
    return hseq, new_h, new_c


def kernel(**inputs):
    return _run(inputs, S_FULL)


# revision 9
# speedup vs baseline: 1.1036x; 1.1036x over previous
"""2-layer LSTM (S=256, B=32, D=H=1024) on 8 trn2 NeuronCores.

Strategy: tensor-parallel over the hidden dim. Core j owns H-slice
[128j:128(j+1)) and, for each layer, the four gate column blocks
(f, i, o, g) of that slice -> 512 local gate columns.

Per layer:
  Phase A/C: xproj = x @ Wx + b as a big GEMM (tokens x 512 per core),
  kept resident in SBUF (bf16).
  Phase B/D: 256 sequential steps. Each step: 8 K-chunk matmuls
  (lhsT = gathered h^T chunks [128,32] bf16, rhs = Wh slice [128,512] bf16)
  accumulate gates in PSUM; DVE adds xproj; ACT sigmoid/tanh; DVE cell
  update (f32); PE-transpose of the local h chunk [32,128]->[128,32];
  AllGather of bf16 chunks across the 8 cores -> next step's h^T.

Layouts: "natural" batch-on-partitions for gates/cell; gathered h^T is
feature-on-partitions, which is exactly the lhsT the next matmul needs.
"""

import numpy as np
import ml_dtypes

S_FULL = 256
B = 32
D = 1024
H = 1024
NC = 8
HL = H // NC          # 128 hidden rows per core
NG = 4 * HL           # 512 local gate columns
KC = D // 128         # 8 contraction chunks

_BF16 = ml_dtypes.bfloat16


def _build(nc_mod, steps):
    import concourse.mybir as mybir
    import concourse.tile as tile

    nc = nc_mod
    f32 = mybir.dt.float32
    bf16 = mybir.dt.bfloat16
    AF = mybir.ActivationFunctionType
    NTOK = steps * B
    MT = NTOK // 128       # token m-tiles (steps/4)

    # ---- DRAM I/O ----
    xT = nc.dram_tensor("xT", [D, NTOK], bf16, kind="ExternalInput")
    wx1 = nc.dram_tensor("wx1", [D, NG], bf16, kind="ExternalInput")
    wx2 = nc.dram_tensor("wx2", [H, NG], bf16, kind="ExternalInput")
    wh1 = nc.dram_tensor("wh1", [H, NG], bf16, kind="ExternalInput")
    wh2 = nc.dram_tensor("wh2", [H, NG], bf16, kind="ExternalInput")
    b1 = nc.dram_tensor("b1", [128, NG], f32, kind="ExternalInput")
    b2 = nc.dram_tensor("b2", [128, NG], f32, kind="ExternalInput")
    ident = nc.dram_tensor("ident", [32, 32], f32, kind="ExternalInput")
    hseq = nc.dram_tensor("hseq", [steps, B, HL], f32, kind="ExternalOutput")
    hcfin = nc.dram_tensor("hcfin", [4, B, HL], f32, kind="ExternalOutput")

    with tile.TileContext(nc) as tc:
        with (
            tc.tile_pool(name="weights", bufs=1) as wpool,
            tc.tile_pool(name="xp", bufs=1) as xppool,
            tc.tile_pool(name="stage", bufs=3) as stpool,
            tc.tile_pool(name="gather", bufs=3) as gpool,
            tc.tile_pool(name="ew", bufs=3) as ewpool,
            tc.tile_pool(name="state", bufs=1) as cpool,
            tc.tile_pool(name="psA", bufs=2, space="PSUM") as psapool,
            tc.tile_pool(name="psG", bufs=2, space="PSUM") as psgpool,
            tc.tile_pool(name="psT", bufs=2, space="PSUM") as pstpool,
            tc.tile_pool(name="dram", bufs=3, space="DRAM") as dpool,
        ):
            # persistent SBUF
            xp_sb = xppool.tile([128, MT * NG], bf16, tag="xp")
            wh1_sb = wpool.tile([128, KC * NG], bf16, tag="wh1")
            wh2_sb = wpool.tile([128, KC * NG], bf16, tag="wh2")
            ident_sb = wpool.tile([32, 32], f32, tag="ident")
            nc.sync.dma_start(ident_sb[:], ident[:])
            nc.sync.dma_start(
                wh1_sb[:].rearrange("p (k g) -> p k g", k=KC),
                wh1.rearrange("(k p) g -> p k g", p=128),
            )
            nc.sync.dma_start(
                wh2_sb[:].rearrange("p (k g) -> p k g", k=KC),
                wh2.rearrange("(k p) g -> p k g", p=128),
            )
            h1T_dram = dpool.tile([KC * 128, NTOK], bf16, tag="h1T")

            def xproj_phase(w_dram, b_dram, src_view):
                """GEMM: xp_sb[:, m*NG:(m+1)*NG] = src^T-tiles @ w + b."""
                w_sb = wpool.tile([128, KC * NG], bf16, tag="wx")
                b_sb = wpool.tile([128, NG], f32, tag="bias")
                nc.sync.dma_start(
                    w_sb[:].rearrange("p (k g) -> p k g", k=KC),
                    w_dram.rearrange("(k p) g -> p k g", p=128),
                )
                nc.sync.dma_start(b_sb[:], b_dram[:])
                for m in range(MT):
                    a_sb = stpool.tile([128, KC * 128], bf16, tag="astage")
                    nc.sync.dma_start(
                        a_sb[:].rearrange("p (k n) -> p k n", k=KC),
                        src_view[:, :, m * 128:(m + 1) * 128],
                    )
                    ps = psapool.tile([128, NG], f32, tag="psA")
                    for k in range(KC):
                        nc.tensor.matmul(
                            ps[:],
                            a_sb[:, k * 128:(k + 1) * 128],
                            w_sb[:, k * NG:(k + 1) * NG],
                            start=(k == 0),
                            stop=(k == KC - 1),
                        )
                    nc.vector.tensor_add(
                        xp_sb[:, m * NG:(m + 1) * NG], ps[:], b_sb[:]
                    )

            def recurrent_phase(layer, wh_sb):
                c_sb = cpool.tile([B, HL], f32, tag=f"c{layer}")
                nc.vector.memset(c_sb[:], 0.0)
                hT_cur = gpool.tile([128, KC * 32], bf16, tag="hT")
                nc.vector.memset(hT_cur[:], 0.0)
                h_last = None
                for t in range(steps):
                    m, r = t // 4, t % 4
                    # gates = h_{t-1} @ Wh  (8 K-chunk matmuls)
                    ps_g = psgpool.tile([B, NG], f32, tag="psG")
                    for k in range(KC):
                        nc.tensor.matmul(
                            ps_g[:],
                            hT_cur[:, k * 32:(k + 1) * 32],
                            wh_sb[:, k * NG:(k + 1) * NG],
                            start=(k == 0),
                            stop=(k == KC - 1),
                        )
                    gates = ewpool.tile([B, NG], f32, tag="gates")
                    nc.vector.tensor_add(
                        gates[:],
                        ps_g[:],
                        xp_sb[32 * r:32 * r + 32, m * NG:(m + 1) * NG],
                    )
                    act = ewpool.tile([B, NG], f32, tag="act")
                    nc.scalar.activation(act[:, 0:384], gates[:, 0:384], AF.Sigmoid)
                    nc.scalar.activation(act[:, 384:512], gates[:, 384:512], AF.Tanh)
                    # f=act[:,0:128] i=act[:,128:256] o=act[:,256:384] g=act[:,384:512]
                    t1 = ewpool.tile([B, HL], f32, tag="t1")
                    t2 = ewpool.tile([B, HL], f32, tag="t2")
                    nc.vector.tensor_mul(t1[:], act[:, 0:128], c_sb[:])
                    nc.vector.tensor_mul(t2[:], act[:, 128:256], act[:, 384:512])
                    nc.vector.tensor_add(c_sb[:], t1[:], t2[:])
                    ct = ewpool.tile([B, HL], f32, tag="ct")
                    nc.scalar.activation(ct[:], c_sb[:], AF.Tanh)
                    h = ewpool.tile([B, HL], f32, tag="h")
                    nc.vector.tensor_mul(h[:], act[:, 256:384], ct[:])
                    h_last = h
                    if layer == 2:
                        nc.sync.dma_start(hseq[t], h[:])
                    last = t == steps - 1
                    if layer == 2 and last:
                        break  # h2 of the final step feeds nothing downstream
                    # h^T chunk -> all-gather -> next step's lhsT chunks
                    tr = pstpool.tile([128, 32], f32, tag="psT")
                    nc.tensor.transpose(tr[:], h[:], ident_sb[:])
                    hTb = ewpool.tile([128, 32], bf16, tag="hTb")
                    nc.vector.tensor_copy(hTb[:], tr[:])
                    bounce = dpool.tile([128, 32], bf16, tag="bounce")
                    gath = dpool.tile([NC * 128, 32], bf16, tag="gath")
                    nc.sync.dma_start(bounce[:], hTb[:])
                    nc.gpsimd.collective_compute(
                        "AllGather",
                        mybir.AluOpType.bypass,
                        replica_groups=[list(range(NC))],
                        ins=[bounce[:].opt()],
                        outs=[gath[:].opt()],
                    )
                    if layer == 1:
                        nc.sync.dma_start(
                            h1T_dram[:].rearrange("(k p) n -> k p n", p=128)[
                                :, :, 32 * t:32 * t + 32
                            ],
                            gath[:].rearrange("(k p) b -> k p b", p=128),
                        )
                    if not last:
                        hT_next = gpool.tile([128, KC * 32], bf16, tag="hT")
                        nc.sync.dma_start(
                            hT_next[:].rearrange("p (k b) -> p k b", k=KC),
                            gath[:].rearrange("(k p) b -> p k b", p=128),
                        )
                        hT_cur = hT_next
                # finals: h / c of the last step
                base = 0 if layer == 1 else 2
                nc.sync.dma_start(hcfin[base], h_last[:])
                nc.sync.dma_start(hcfin[base + 1], c_sb[:])

            # layer 1
            xproj_phase(wx1, b1, xT.rearrange("(k p) n -> p k n", p=128))
            recurrent_phase(1, wh1_sb)
            # layer 2
            xproj_phase(
                wx2, b2, h1T_dram[:].rearrange("(k p) n -> p k n", p=128)
            )
            recurrent_phase(2, wh2_sb)

    return nc


def _build_merged(nc_mod, steps):
    """v2: layer-pipelined. Tick t runs L1 step t and L2 step t-1; ONE
    AllGather per tick ships [h1_t ; h2_{t-1}] (256 rows x 32 bf16).
    L2's 16 matmuls depend only on the previous tick's gather, so they
    execute during the current tick's AllGather wait (PE stays warm).
    """
    import concourse.mybir as mybir
    import concourse.tile as tile

    nc = nc_mod
    f32 = mybir.dt.float32
    bf16 = mybir.dt.bfloat16
    AF = mybir.ActivationFunctionType
    NTOK = steps * B
    MT = NTOK // 128

    xT = nc.dram_tensor("xT", [D, NTOK], bf16, kind="ExternalInput")
    wx1 = nc.dram_tensor("wx1", [D, NG], bf16, kind="ExternalInput")
    wx2 = nc.dram_tensor("wx2", [H, NG], bf16, kind="ExternalInput")
    wh1 = nc.dram_tensor("wh1", [H, NG], bf16, kind="ExternalInput")
    wh2 = nc.dram_tensor("wh2", [H, NG], bf16, kind="ExternalInput")
    b1 = nc.dram_tensor("b1", [128, NG], f32, kind="ExternalInput")
    b2 = nc.dram_tensor("b2", [128, NG], f32, kind="ExternalInput")
    ident = nc.dram_tensor("ident", [32, 32], f32, kind="ExternalInput")
    hseq = nc.dram_tensor("hseq", [steps, B, HL], f32, kind="ExternalOutput")
    hcfin = nc.dram_tensor("hcfin", [4, B, HL], f32, kind="ExternalOutput")

    with tile.TileContext(nc) as tc:
        with (
            tc.tile_pool(name="weights", bufs=1) as wpool,
            tc.tile_pool(name="xp", bufs=1) as xppool,
            tc.tile_pool(name="stage", bufs=3) as stpool,
            tc.tile_pool(name="gather", bufs=3) as gpool,
            tc.tile_pool(name="ew", bufs=3) as ewpool,
            tc.tile_pool(name="state", bufs=1) as cpool,
            tc.tile_pool(name="psA", bufs=2, space="PSUM") as psapool,
            tc.tile_pool(name="psG", bufs=3, space="PSUM") as psgpool,
            tc.tile_pool(name="psT", bufs=3, space="PSUM") as pstpool,
            tc.tile_pool(name="dram", bufs=3, space="DRAM") as dpool,
        ):
            xp_sb = xppool.tile([128, MT * NG], bf16, tag="xp")
            wh1_sb = wpool.tile([128, KC * NG], bf16, tag="wh1")
            wh2_sb = wpool.tile([128, KC * NG], bf16, tag="wh2")
            wx2_sb = wpool.tile([128, KC * NG], bf16, tag="wx2")
            b2_sb = wpool.tile([128, NG], f32, tag="b2")
            ident_sb = wpool.tile([32, 32], f32, tag="ident")
            nc.sync.dma_start(ident_sb[:], ident[:])
            for w_sb, w_dr in ((wh1_sb, wh1), (wh2_sb, wh2), (wx2_sb, wx2)):
                nc.sync.dma_start(
                    w_sb[:].rearrange("p (k g) -> p k g", k=KC),
                    w_dr.rearrange("(k p) g -> p k g", p=128),
                )
            nc.sync.dma_start(b2_sb[:], b2[:])

            # ---- phase A: xproj for layer 1 ----
            wxa_sb = wpool.tile([128, KC * NG], bf16, tag="wx1")
            b1_sb = wpool.tile([128, NG], f32, tag="b1")
            nc.sync.dma_start(
                wxa_sb[:].rearrange("p (k g) -> p k g", k=KC),
                wx1.rearrange("(k p) g -> p k g", p=128),
            )
            nc.sync.dma_start(b1_sb[:], b1[:])
            xTv = xT.rearrange("(k p) n -> p k n", p=128)
            for m in range(MT):
                a_sb = stpool.tile([128, KC * 128], bf16, tag="astage")
                nc.sync.dma_start(
                    a_sb[:].rearrange("p (k n) -> p k n", k=KC),
                    xTv[:, :, m * 128:(m + 1) * 128],
                )
                ps = psapool.tile([128, NG], f32, tag="psA")
                for k in range(KC):
                    nc.tensor.matmul(
                        ps[:],
                        a_sb[:, k * 128:(k + 1) * 128],
                        wxa_sb[:, k * NG:(k + 1) * NG],
                        start=(k == 0),
                        stop=(k == KC - 1),
                    )
                nc.vector.tensor_add(xp_sb[:, m * NG:(m + 1) * NG], ps[:], b1_sb[:])

            # ---- pipelined ticks ----
            c1_sb = cpool.tile([B, HL], f32, tag="c1")
            c2_sb = cpool.tile([B, HL], f32, tag="c2")
            nc.vector.memset(c1_sb[:], 0.0)
            nc.vector.memset(c2_sb[:], 0.0)
            zero_sb = wpool.tile([128, 32], bf16, tag="zero")
            nc.vector.memset(zero_sb[:], 0.0)

            # gathered layout per tick: [128, KC, 2, 32]; l=0 -> h1^T, l=1 -> h2^T
            hT_prev = gpool.tile([128, KC * 2 * 32], bf16, tag="hT")
            nc.vector.memset(hT_prev[:], 0.0)
            bounce_cur = dpool.tile([2 * 128, 32], bf16, tag="bounce")
            nc.sync.dma_start(bounce_cur[128:256, :], zero_sb[:])  # h2_{-1} = 0

            def ew_and_transpose(ps_g, xp_ap, c_sb, layer):
                """gates -> (h tile [32,128] f32, hT bf16 [128,32])."""
                gates = ewpool.tile([B, NG], f32, tag=f"gates{layer}")
                nc.vector.tensor_add(gates[:], ps_g[:], xp_ap)
                act = ewpool.tile([B, NG], f32, tag=f"act{layer}")
                nc.scalar.activation(act[:, 0:384], gates[:, 0:384], AF.Sigmoid)
                nc.scalar.activation(act[:, 384:512], gates[:, 384:512], AF.Tanh)
                t1 = ewpool.tile([B, HL], f32, tag=f"t1_{layer}")
                t2 = ewpool.tile([B, HL], f32, tag=f"t2_{layer}")
                nc.vector.tensor_mul(t1[:], act[:, 0:128], c_sb[:])
                nc.vector.tensor_mul(t2[:], act[:, 128:256], act[:, 384:512])
                nc.vector.tensor_add(c_sb[:], t1[:], t2[:])
                ct = ewpool.tile([B, HL], f32, tag=f"ct{layer}")
                nc.scalar.activation(ct[:], c_sb[:], AF.Tanh)
                h = ewpool.tile([B, HL], f32, tag=f"h{layer}")
                nc.vector.tensor_mul(h[:], act[:, 256:384], ct[:])
                tr = pstpool.tile([128, 32], f32, tag="psT")
                nc.tensor.transpose(tr[:], h[:], ident_sb[:])
                hTb = ewpool.tile([128, 32], bf16, tag=f"hTb{layer}")
                nc.vector.tensor_copy(hTb[:], tr[:])
                return h, hTb

            h1_last = h2_last = None
            for t in range(steps + 1):
                # --- L1 step t ---
                if t < steps:
                    m, r = t // 4, t % 4
                    ps1 = psgpool.tile([B, NG], f32, tag="psG")
                    for k in range(KC):
                        nc.tensor.matmul(
                            ps1[:],
                            hT_prev[:, k * 64:k * 64 + 32],
                            wh1_sb[:, k * NG:(k + 1) * NG],
                            start=(k == 0),
                            stop=(k == KC - 1),
                        )
                    h1, hTb1 = ew_and_transpose(
                        ps1,
                        xp_sb[32 * r:32 * r + 32, m * NG:(m + 1) * NG],
                        c1_sb,
                        1,
                    )
                    h1_last = h1
                    nc.sync.dma_start(bounce_cur[0:128, :], hTb1[:])
                # --- AllGather tick t (ships [h1_t, h2_{t-2}]) ---
                gath = dpool.tile([NC * 2 * 128, 32], bf16, tag="gath")
                nc.gpsimd.collective_compute(
                    "AllGather",
                    mybir.AluOpType.bypass,
                    replica_groups=[list(range(NC))],
                    ins=[bounce_cur[:].opt()],
                    outs=[gath[:].opt()],
                )
                hT_cur = gpool.tile([128, KC * 2 * 32], bf16, tag="hT")
                nc.sync.dma_start(
                    hT_cur[:].rearrange("p (k l b) -> p k l b", k=KC, l=2),
                    gath[:].rearrange("(k l p) b -> p k l b", l=2, p=128),
                )
                # --- L2 step t-1: h1-matmuls use last tick's gather (run
                # during this tick's AG); h2-matmuls use this tick's ---
                if t >= 1:
                    ps2 = psgpool.tile([B, NG], f32, tag="psG")
                    for k in range(KC):
                        nc.tensor.matmul(
                            ps2[:],
                            hT_prev[:, k * 64:k * 64 + 32],  # h1_{t-1}
                            wx2_sb[:, k * NG:(k + 1) * NG],
                            start=(k == 0),
                            stop=False,
                        )
                    for k in range(KC):
                        nc.tensor.matmul(
                            ps2[:],
                            hT_cur[:, k * 64 + 32:k * 64 + 64],  # h2_{t-2}
                            wh2_sb[:, k * NG:(k + 1) * NG],
                            start=False,
                            stop=(k == KC - 1),
                        )
                    h2, hTb2 = ew_and_transpose(ps2, b2_sb[0:B, :], c2_sb, 2)
                    h2_last = h2
                    nc.sync.dma_start(hseq[t - 1], h2[:])
                    if t < steps:
                        bounce_next = dpool.tile([2 * 128, 32], bf16, tag="bounce")
                        nc.sync.dma_start(bounce_next[128:256, :], hTb2[:])
                        bounce_cur = bounce_next
                hT_prev = hT_cur

            nc.sync.dma_start(hcfin[0], h1_last[:])
            nc.sync.dma_start(hcfin[1], c1_sb[:])
            nc.sync.dma_start(hcfin[2], h2_last[:])
            nc.sync.dma_start(hcfin[3], c2_sb[:])

    return nc


def _run(inputs, steps, version=2):
    import concourse.bacc as bacc
    from concourse.bass_utils import run_bass_kernel_spmd

    x = np.asarray(inputs["x"], np.float32)[:steps]
    Wx = np.asarray(inputs["Wx"], np.float32)
    Wh = np.asarray(inputs["Wh"], np.float32)
    b = np.asarray(inputs["b"], np.float32)

    xT = np.ascontiguousarray(
        x.reshape(steps * B, D).T
    ).astype(_BF16)
    ident = np.eye(32, dtype=np.float32)

    in_maps = []
    for j in range(NC):
        # local gate column order: f, i, o, g  (reference order: f, i, g, o)
        idx = np.concatenate(
            [np.arange(g * H + HL * j, g * H + HL * (j + 1)) for g in (0, 1, 3, 2)]
        )
        m = {
            "xT": xT,
            "wx1": np.ascontiguousarray(Wx[0][:, idx]).astype(_BF16),
            "wx2": np.ascontiguousarray(Wx[1][:, idx]).astype(_BF16),
            "wh1": np.ascontiguousarray(Wh[0][:, idx]).astype(_BF16),
            "wh2": np.ascontiguousarray(Wh[1][:, idx]).astype(_BF16),
            "b1": np.ascontiguousarray(
                np.broadcast_to(b[0][idx], (128, NG))
            ).astype(np.float32),
            "b2": np.ascontiguousarray(
                np.broadcast_to(b[1][idx], (128, NG))
            ).astype(np.float32),
            "ident": ident,
        }
        in_maps.append(m)

    nc = bacc.Bacc("TRN2", target_bir_lowering=False, debug=False, num_devices=NC)
    if version == 2:
        _build_merged(nc, steps)
    else:
        _build(nc, steps)
    nc.compile()
    res = run_bass_kernel_spmd(nc, in_maps, core_ids=list(range(NC)))

    hseq = np.concatenate(
        [res.results[j]["hseq"] for j in range(NC)], axis=2
    ).astype(np.float32)
    hc = [res.results[j]["hcfin"] for j in range(NC)]
    new_h = np.stack(
        [
            np.concatenate([hc[j][0] for j in range(NC)], axis=1),
            np.concatenate([hc[j][2] for j in range(NC)], axis=1),
        ]
    ).astype(np.float32)
    new_c = np.stack(
        [
            np.concatenate([hc[j][1] for j in range(NC)], axis=1),
            np.concatenate([hc[j][3] for j in range(NC)], axis=1),
        ]
    ).astype(np.float32)
    return hseq, new_h, new_c


def kernel(**inputs):
    return _run(inputs, S_FULL)
